# revision 1
# baseline (speedup 1.0000x reference)
"""GAT (graph attention) Bass kernel for Trainium2, 8-core SPMD.

Strategy: edge-parallel with receiver-range sharding. Host sorts edges by
receiver and packs receivers into fixed-capacity "windows" (<=127 nodes,
<=G*128 edges). Each core processes an equal number of windows; the segment
softmax and weighted segment-sum are fully core-local (no collectives).

Device kernel, per core:
  phase A: hs = [x@W | x@W@A1 | x@W@A2]  (A1/A2 embed the per-head attention
           vectors) -> DRAM tables hs[N,68] (h|s1) and s2[N,4].
  phase B: per window, indirect-DMA gathers of the 2048 edge rows (by sender
           for h|s1, by receiver for s2), logits -> LeakyReLU -> exp on the
           scalar engine, feature scaling + one-hot build on the vector
           engine, 16 accumulating 128-contraction matmuls into PSUM
           (segment sum of both softmax numerator and denominator), then a
           reciprocal multiply and a contiguous DMA to a staged output.

Host reassembles the staged windows into the full [N, H*U] output.
"""

import os
import sys

import numpy as np

for _p in ("/opt/trn_rl_repo", os.path.expanduser("~/.axon_site/_ro/trn_rl_repo")):
    if os.path.isdir(_p) and _p not in sys.path:
        sys.path.insert(0, _p)

P = 128          # partitions / PE contraction
G = 16           # edge groups per window (window = G*128 edge slots)
WIN_EDGES = G * P
WIN_NODES = 127  # real receiver rows per window; row 127 collects pad edges
HEADS = 4
UNITS = 16
HU = HEADS * UNITS          # 64
HS_COLS = HU + HEADS        # 68: h | s1
LEAKY_ALPHA = 0.2
XTILE = 512                 # phase-A node tile
ABLATE = "full"             # dev-only: "phaseA" | "nocompute" | "nogather"
REPS = 1                    # dev-only: replicate kernel body for timing


def _pack_windows(rcv_sorted, order, n_nodes):
    """Pack receivers (ascending) into windows of <=WIN_NODES nodes and
    <=WIN_EDGES edges."""
    deg = np.bincount(rcv_sorted, minlength=n_nodes)
    starts = np.concatenate(([0], np.cumsum(deg)))
    windows = []
    n = 0
    while n < n_nodes:
        n0 = n
        e0 = starts[n]
        while (
            n < n_nodes
            and (n - n0) < WIN_NODES
            and (starts[n + 1] - e0) <= WIN_EDGES
        ):
            n += 1
        assert n > n0, f"node {n} degree {deg[n]} exceeds window capacity"
        windows.append((n0, n - n0, e0, starts[n]))
    return windows


def _build_host_data(x, edge_index, W, att_w1, att_w2, n_cores):
    n_nodes, in_feat = x.shape
    snd = edge_index[:, 0].astype(np.int64)
    rcv = edge_index[:, 1].astype(np.int64)

    order = np.argsort(rcv, kind="stable")
    rcv_sorted = rcv[order]
    windows = _pack_windows(rcv_sorted, order, n_nodes)

    nw_total = len(windows)
    nw = -(-nw_total // n_cores)  # windows per core, padded
    n_win_padded = nw * n_cores

    # per-window device metadata, edge slot q=(p*G+j) <- position q of the
    # window's (padded) edge list
    snd_idx = np.zeros((n_win_padded, P, G), dtype=np.int32)
    s2_idx = np.zeros((n_win_padded, P, G), dtype=np.int32)
    rcv_loc = np.full((n_win_padded, P, G), float(P - 1), dtype=np.float32)

    for w, (n0, cnt, e0, e1) in enumerate(windows):
        ne = e1 - e0
        eidx = order[e0:e1]
        buf_s = np.zeros(WIN_EDGES, dtype=np.int32)
        buf_r = np.full(WIN_EDGES, float(P - 1), dtype=np.float32)
        buf_v = np.zeros(WIN_EDGES, dtype=np.int32)
        buf_s[:ne] = snd[eidx]
        buf_r[:ne] = (rcv_sorted[e0:e1] - n0).astype(np.float32)
        buf_v[:ne] = rcv_sorted[e0:e1]
        snd_idx[w] = buf_s.reshape(P, G)
        rcv_loc[w] = buf_r.reshape(P, G)
        s2_idx[w] = buf_v.reshape(P, G)

    # attention vectors as [HU, 2H]: A[h*U+u, h] = att_w1[h,0,u]; +H col for w2
    A12 = np.zeros((HU, 2 * HEADS), dtype=np.float32)
    for h in range(HEADS):
        A12[h * UNITS:(h + 1) * UNITS, h] = att_w1[h, 0]
        A12[h * UNITS:(h + 1) * UNITS, HEADS + h] = att_w2[h, 0]

    npad = -(-n_nodes // XTILE) * XTILE
    xT = np.zeros((in_feat, npad), dtype=np.float32)
    xT[:, :n_nodes] = np.ascontiguousarray(x.T)

    iota = np.broadcast_to(np.arange(P, dtype=np.float32), (P, P)).copy()
    identity = np.eye(P, dtype=np.float32)

    host = {
        "windows": windows,
        "nw": nw,
        "npad": npad,
        "deg": np.bincount(rcv, minlength=n_nodes),
    }
    per_core = []
    for c in range(n_cores):
        w0 = c * nw
        per_core.append({
            "xT": xT,
            "W": np.ascontiguousarray(W.astype(np.float32)),
            "A12": A12,
            "iota": iota,
            "identity": identity,
            "snd_idx": np.ascontiguousarray(
                snd_idx[w0:w0 + nw].transpose(1, 0, 2).reshape(P, nw * G)),
            "rcv_loc": np.ascontiguousarray(
                rcv_loc[w0:w0 + nw].transpose(1, 0, 2).reshape(P, nw * G)),
            "s2_idx": np.ascontiguousarray(
                s2_idx[w0:w0 + nw].transpose(1, 0, 2).reshape(P, nw * G)),
        })
    return host, per_core


def _build_bass(n_nodes, npad, nw, in_feat):
    from concourse import bacc, mybir, tile
    import concourse.bass as bass

    f32 = mybir.dt.float32
    i32 = mybir.dt.int32

    nc = bacc.Bacc("TRN2", target_bir_lowering=False, debug=False,
                   enable_asserts=False, num_devices=1)

    xT_d = nc.dram_tensor("xT", [in_feat, npad], f32, kind="ExternalInput").ap()
    W_d = nc.dram_tensor("W", [in_feat, HU], f32, kind="ExternalInput").ap()
    A12_d = nc.dram_tensor("A12", [HU, 2 * HEADS], f32, kind="ExternalInput").ap()
    iota_d = nc.dram_tensor("iota", [P, P], f32, kind="ExternalInput").ap()
    ident_d = nc.dram_tensor("identity", [P, P], f32, kind="ExternalInput").ap()
    snd_d = nc.dram_tensor("snd_idx", [P, nw * G], i32, kind="ExternalInput").ap()
    rcvl_d = nc.dram_tensor("rcv_loc", [P, nw * G], f32, kind="ExternalInput").ap()
    s2i_d = nc.dram_tensor("s2_idx", [P, nw * G], i32, kind="ExternalInput").ap()

    out_d = nc.dram_tensor("staged", [nw * P, HU], f32, kind="ExternalOutput").ap()

    hs_d = nc.dram_tensor("hs_tab", [npad, HS_COLS], f32, kind="Internal").ap()
    s2_d = nc.dram_tensor("s2_tab", [npad, HEADS], f32, kind="Internal").ap()

    ntiles = npad // XTILE

    with tile.TileContext(nc) as tc:
        with tc.tile_pool(name="consts", bufs=1) as cpool:
            W_sb = cpool.tile([in_feat, HU], f32, tag="w")
            nc.sync.dma_start(out=W_sb[:], in_=W_d[:])
            A12_sb = cpool.tile([HU, 2 * HEADS], f32, tag="a12")
            nc.sync.dma_start(out=A12_sb[:], in_=A12_d[:])
            iota_sb = cpool.tile([P, P], f32, tag="iota")
            nc.sync.dma_start(out=iota_sb[:], in_=iota_d[:])
            id_sb = cpool.tile([P, P], f32, tag="ident")
            nc.sync.dma_start(out=id_sb[:], in_=ident_d[:])
            snd_sb = cpool.tile([P, nw * G], i32, tag="snd")
            nc.sync.dma_start(out=snd_sb[:], in_=snd_d[:])
            rcvl_sb = cpool.tile([P, nw * G], f32, tag="rcvl")
            nc.sync.dma_start(out=rcvl_sb[:], in_=rcvl_d[:])
            s2i_sb = cpool.tile([P, nw * G], i32, tag="s2i")
            nc.sync.dma_start(out=s2i_sb[:], in_=s2i_d[:])
            wcat_sb = cpool.tile([in_feat, HU + 2 * HEADS], f32, tag="wcat")

            # fold attention vectors: WA = W @ A12 (needs W^T as lhsT)
            with tc.tile_pool(name="p0psum", bufs=1, space="PSUM") as p0:
                wt_ps = p0.tile([HU, in_feat], f32, tag="wt")
                nc.tensor.transpose(out=wt_ps[:], in_=W_sb[:], identity=id_sb[:])
                wt_sb = cpool.tile([HU, in_feat], f32, tag="wtsb")
                nc.vector.tensor_copy(out=wt_sb[:], in_=wt_ps[:])
                wa_ps = p0.tile([in_feat, 2 * HEADS], f32, tag="wa")
                nc.tensor.matmul(out=wa_ps[:], lhsT=wt_sb[:], rhs=A12_sb[:],
                                 start=True, stop=True)
                nc.vector.tensor_copy(out=wcat_sb[:, HU:], in_=wa_ps[:])
                nc.vector.tensor_copy(out=wcat_sb[:, :HU], in_=W_sb[:])

            # ---- phase A: hs tables ----
            nblk = XTILE // P
            wc = HU + 2 * HEADS  # 72
            with tc.tile_pool(name="pa_x", bufs=3) as pax, \
                 tc.tile_pool(name="pa_ps", bufs=2, space="PSUM") as paps, \
                 tc.tile_pool(name="pa_hs", bufs=3) as pahs:
              for _rep in range(REPS):
                for t in range(ntiles):
                    xt = pax.tile([in_feat, XTILE], f32, tag="xt")
                    nc.sync.dma_start(
                        out=xt[:], in_=xT_d[:, t * XTILE:(t + 1) * XTILE])
                    ps = paps.tile([P, nblk * wc], f32, tag="ps")
                    for i in range(nblk):
                        nc.tensor.matmul(
                            out=ps[:, i * wc:(i + 1) * wc],
                            lhsT=xt[:, i * P:(i + 1) * P],
                            rhs=wcat_sb[:], start=True, stop=True)
                    hsb = pahs.tile([P, nblk * wc], f32, tag="hsb")
                    nc.vector.tensor_copy(out=hsb[:], in_=ps[:])
                    hsb3 = hsb[:].rearrange("p (i c) -> p i c", c=wc)
                    dst_hs = hs_d[t * XTILE:(t + 1) * XTILE, :].rearrange(
                        "(i p) c -> p i c", p=P)
                    nc.sync.dma_start(out=dst_hs, in_=hsb3[:, :, 0:HS_COLS])
                    dst_s2 = s2_d[t * XTILE:(t + 1) * XTILE, :].rearrange(
                        "(i p) c -> p i c", p=P)
                    nc.sync.dma_start(out=dst_s2, in_=hsb3[:, :, HS_COLS:wc])

            # ---- phase B: windows ----
            with tc.tile_pool(name="pb_g", bufs=2) as pbg, \
                 tc.tile_pool(name="pb_sm", bufs=2) as pbsm, \
                 tc.tile_pool(name="pb_oh", bufs=2) as pboh, \
                 tc.tile_pool(name="pb_ps", bufs=2, space="PSUM") as pbps, \
                 tc.tile_pool(name="pb_out", bufs=2) as pbout:
              for _rep in range(REPS):
                for w in range(nw if ABLATE != "phaseA" else 0):
                    cs = slice(w * G, (w + 1) * G)
                    hs_g = pbg.tile([P, G * HS_COLS], f32, tag="hsg")
                    s2_g = pbsm.tile([P, G * HEADS], f32, tag="s2g")
                    if ABLATE != "nogather":
                        for j in range(G):
                            col = w * G + j
                            nc.gpsimd.indirect_dma_start(
                                out=hs_g[:, j * HS_COLS:(j + 1) * HS_COLS],
                                out_offset=None, in_=hs_d[:],
                                in_offset=bass.IndirectOffsetOnAxis(
                                    ap=snd_sb[:, col:col + 1], axis=0))
                            nc.gpsimd.indirect_dma_start(
                                out=s2_g[:, j * HEADS:(j + 1) * HEADS],
                                out_offset=None, in_=s2_d[:],
                                in_offset=bass.IndirectOffsetOnAxis(
                                    ap=s2i_sb[:, col:col + 1], axis=0))
                    if ABLATE == "nocompute":
                        continue

                    hs_g3 = hs_g[:].rearrange("p (j c) -> p j c", c=HS_COLS)
                    logit = pbsm.tile([P, G * HEADS], f32, tag="logit")
                    lg3 = logit[:].rearrange("p (j h) -> p j h", h=HEADS)
                    nc.vector.tensor_add(
                        out=lg3, in0=hs_g3[:, :, HU:HS_COLS],
                        in1=s2_g[:].rearrange("p (j h) -> p j h", h=HEADS))
                    neg = pbsm.tile([P, G * HEADS], f32, tag="neg")
                    nc.vector.tensor_scalar(
                        out=neg[:], in0=logit[:], scalar1=0.0,
                        scalar2=LEAKY_ALPHA, op0=mybir.AluOpType.min,
                        op1=mybir.AluOpType.mult)
                    lrl = pbsm.tile([P, G * HEADS], f32, tag="lrl")
                    nc.vector.scalar_tensor_tensor(
                        out=lrl[:], in0=logit[:], scalar=0.0, in1=neg[:],
                        op0=mybir.AluOpType.max, op1=mybir.AluOpType.add)
                    expo = pbsm.tile([P, G * HEADS], f32, tag="expo")
                    nc.scalar.activation(
                        out=expo[:], in_=lrl[:],
                        func=mybir.ActivationFunctionType.Exp)

                    rhs = pbg.tile([P, G * HS_COLS], f32, tag="rhs")
                    rhs3 = rhs[:].rearrange("p (j c) -> p j c", c=HS_COLS)
                    ex3 = expo[:].rearrange("p (j h) -> p j h", h=HEADS)
                    nc.vector.tensor_tensor(
                        out=rhs3[:, :, 0:HU].rearrange("p j (h u) -> p j h u",
                                                       u=UNITS),
                        in0=hs_g3[:, :, 0:HU].rearrange("p j (h u) -> p j h u",
                                                        u=UNITS),
                        in1=ex3.broadcast_to([P, G, HEADS, UNITS]),
                        op=mybir.AluOpType.mult)
                    nc.vector.tensor_copy(out=rhs3[:, :, HU:HS_COLS], in_=ex3)

                    onehot = pboh.tile([P, G * P], f32, tag="oh")
                    oh3 = onehot[:].rearrange("p (j c) -> p j c", c=P)
                    nc.vector.tensor_tensor(
                        out=oh3,
                        in0=iota_sb[:].broadcast_to([P, P, G]).rearrange(
                            "p c j -> p j c"),
                        in1=rcvl_sb[:, cs].broadcast_to([P, G, P]),
                        op=mybir.AluOpType.is_equal)

                    ps = pbps.tile([P, HS_COLS], f32, tag="acc")
                    for j in range(G):
                        nc.tensor.matmul(
                            out=ps[:],
                            lhsT=onehot[:, j * P:(j + 1) * P],
                            rhs=rhs[:, j * HS_COLS:(j + 1) * HS_COLS],
                            start=(j == 0), stop=(j == G - 1))

                    recip = pbout.tile([P, HEADS], f32, tag="recip")
                    nc.vector.reciprocal(out=recip[:], in_=ps[:, HU:HS_COLS])
                    osb = pbout.tile([P, HU], f32, tag="osb")
                    nc.vector.tensor_tensor(
                        out=osb[:].rearrange("p (h u) -> p h u", u=UNITS),
                        in0=ps[:, 0:HU].rearrange("p (h u) -> p h u", u=UNITS),
                        in1=recip[:].broadcast_to([P, HEADS, UNITS]),
                        op=mybir.AluOpType.mult)
                    nc.sync.dma_start(
                        out=out_d[w * P:(w + 1) * P, :], in_=osb[:])

    nc.compile()
    return nc


def _run(nc, per_core, n_cores):
    from concourse import bass_utils

    want_trace = bool(os.environ.get("GAT_TRACE"))
    res = bass_utils.run_bass_kernel_spmd(
        nc, per_core, core_ids=list(range(n_cores)), trace=want_trace)
    return res


def kernel(x, edge_index, W, att_w1, att_w2, n_cores=8, _return_results=False):
    x = np.asarray(x)
    edge_index = np.asarray(edge_index)
    W = np.asarray(W)
    att_w1 = np.asarray(att_w1)
    att_w2 = np.asarray(att_w2)

    n_nodes, in_feat = x.shape
    host, per_core = _build_host_data(x, edge_index, W, att_w1, att_w2, n_cores)
    nc = _build_bass(n_nodes, host["npad"], host["nw"], in_feat)
    res = _run(nc, per_core, n_cores)

    nw = host["nw"]
    out = np.zeros((n_nodes, HU), dtype=np.float32)
    for w, (n0, cnt, e0, e1) in enumerate(host["windows"]):
        c, s = divmod(w, nw)
        staged = res.results[c]["staged"]
        out[n0:n0 + cnt] = staged[s * P:s * P + cnt]
    out[host["deg"] == 0] = 0.0
    if _return_results:
        return out, res
    return out



# revision 10
# speedup vs baseline: 82.2941x; 82.2941x over previous
"""GAT (graph attention) Bass kernel for Trainium2, 8-core SPMD.

Strategy: edge-parallel with receiver-sorted packed windows, built around
``dma_gather`` (the TIE-accelerated SWDGE gather) instead of per-column
indirect DMAs.

Phase A (per core, replicated): hs table = [x@W/64 | x@W@A1 | x@W@A2] written
to DRAM as fp16 rows of 128 cols (256B — the dma_gather minimum row size),
split into a "lo" table (nodes < 32768) and a "hi" table (the rest) because
gather indices are int16.

Phase B: host packs receiver-sorted edges into windows of <=127 receiver
nodes and B=L+H blocks of 128 edge slots (lo-sender slots first, then
hi-sender slots, each padded to a block boundary).  Per window the kernel
issues three dma_gathers (senders from the lo table, senders from the hi
table, receivers from the window's half), computes per-edge logits ->
leaky-relu -> clamp -> exp on fp16 tiles, scales sender features by the
softmax numerator, and segment-sums numerator and denominator into PSUM with
B one-hot matmuls.  A reciprocal multiply produces the window's output rows.

Windows are dealt to cores round-robin (position s on core c = global window
s*8+c) with the lo-half window count padded to a multiple of n_cores, so the
receiver-gather's table half is uniform across cores at every position —
required because all cores share one compiled program.

The h features are stored as h/64 in the table (exact power-of-2 scale) so
the fp16 product h*exp(logit) cannot overflow; the final reciprocal is
multiplied by 64 to compensate.  Logits are clamped at 10.5 so exp stays
finite in fp16.
"""

import os
import sys

import numpy as np

for _p in ("/opt/trn_rl_repo", os.path.expanduser("~/.axon_site/_ro/trn_rl_repo")):
    if os.path.isdir(_p) and _p not in sys.path:
        sys.path.insert(0, _p)

P = 128
HEADS = 4
UNITS = 16
HU = HEADS * UNITS          # 64
ROW = 128                   # fp16 cols per table row (256 B)
WC = HU + 2 * HEADS         # 72 useful cols: h | s1 | s2
LEAKY_ALPHA = 0.2
CLAMP = 10.5
HSCALE = 64.0               # h stored as h/64 (exact in fp16)
XTILE = 512                 # phase-A node tile
SPLIT = 32768               # lo/hi table split (int16 index range)
G_PACK = 16                 # max total blocks (ceil(nlo/128)+ceil(nhi/128))
WIN_NODES = 127             # receivers per window; local 127 = trash row
REPS = 1                    # dev-only: replicate kernel body for timing
ABLATE = "full"             # dev-only: "phaseA" | "nogather" | "nocompute"


def _pack_windows(lo_deg, hi_deg, n_nodes):
    """Pack nodes (ascending) into windows of <=WIN_NODES receivers with
    ceil(nlo/128)+ceil(nhi/128) <= G_PACK, not straddling SPLIT."""
    windows = []
    n = 0
    while n < n_nodes:
        n0 = n
        nlo = nhi = 0
        in_lo = n0 < SPLIT
        while n < n_nodes and (n - n0) < WIN_NODES:
            if (n < SPLIT) != in_lo:
                break
            nl, nh = nlo + lo_deg[n], nhi + hi_deg[n]
            blocks = -(-nl // P) + (-(-nh // P) if nh else 0)
            if blocks > G_PACK:
                break
            nlo, nhi = nl, nh
            n += 1
        assert n > n0, f"node {n} degree exceeds window capacity"
        windows.append((n0, n - n0, nlo, nhi))
    return windows


def _wrap_idx(idx):
    """dma_gather index layout: position q -> [q%16, q//16], tiled x8 to 128
    partitions.  idx: [n] int16 -> [128, n//16]."""
    return np.tile(idx.reshape(-1, 16).T, (8, 1))


def _build_host_data(x, edge_index, W, att_w1, att_w2, n_cores):
    n_nodes, in_feat = x.shape
    snd = edge_index[:, 0].astype(np.int64)
    rcv = edge_index[:, 1].astype(np.int64)

    order = np.argsort(rcv, kind="stable")
    rs = rcv[order]
    ss = snd[order]
    deg = np.bincount(rs, minlength=n_nodes)
    lo_deg = np.bincount(rs[ss < SPLIT], minlength=n_nodes)
    hi_deg = deg - lo_deg
    starts = np.concatenate(([0], np.cumsum(deg)))

    windows = _pack_windows(lo_deg, hi_deg, n_nodes)
    n_lo_win = sum(1 for w in windows if w[0] < SPLIT)
    n_hi_win = len(windows) - n_lo_win
    n_lo_pad = -(-n_lo_win // n_cores) * n_cores
    n_hi_pad = -(-n_hi_win // n_cores) * n_cores
    nw = (n_lo_pad + n_hi_pad) // n_cores
    lo_pos = n_lo_pad // n_cores          # positions [0, lo_pos) are lo-half

    L = max(-(-w[2] // P) for w in windows)
    H = max(1, max(-(-w[3] // P) for w in windows))
    B = L + H

    # assign[c][s] = global window index or None (padding)
    assign = [[None] * nw for _ in range(n_cores)]
    for i in range(len(windows)):
        if i < n_lo_win:
            s, c = divmod(i, n_cores)
        else:
            j = i - n_lo_win
            s, c = divmod(j, n_cores)
            s += lo_pos
        assign[c][s] = i

    # per-window slot tables (slot q -> partition q%128, block q//128)
    nwin = len(windows)
    snd_lo = np.zeros((nwin, L * P), dtype=np.int16)
    snd_hi = np.zeros((nwin, H * P), dtype=np.int16)
    rcv_ix = np.zeros((nwin, B * P), dtype=np.int16)
    rcv_loc = np.full((nwin, B * P), 255.0, dtype=np.float16)

    for w, (n0, cnt, nlo, nhi) in enumerate(windows):
        e0, e1 = starts[n0], starts[n0 + cnt]
        er, es = rs[e0:e1], ss[e0:e1]
        lo_m = es < SPLIT
        off = 0 if n0 < SPLIT else SPLIT
        s_lo, r_lo = es[lo_m], er[lo_m]
        s_hi, r_hi = es[~lo_m] - SPLIT, er[~lo_m]
        snd_lo[w, :nlo] = s_lo.astype(np.int16)
        snd_hi[w, :nhi] = s_hi.astype(np.int16)
        rcv_ix[w, :nlo] = (r_lo - off).astype(np.int16)
        rcv_ix[w, L * P:L * P + nhi] = (r_hi - off).astype(np.int16)
        rcv_ix[w, nlo:L * P] = np.int16(n0 - off)
        rcv_ix[w, L * P + nhi:] = np.int16(n0 - off)
        rcv_loc[w, :nlo] = (r_lo - n0).astype(np.float16)
        rcv_loc[w, L * P:L * P + nhi] = (r_hi - n0).astype(np.float16)

    empty_sl = np.zeros(L * P, dtype=np.int16)
    empty_sh = np.zeros(H * P, dtype=np.int16)
    empty_rv = np.zeros(B * P, dtype=np.int16)
    empty_rl = np.full((P, B), 255.0, dtype=np.float16)

    # rcv_loc in device layout [P, B] per window
    rcv_loc_dev = rcv_loc.reshape(nwin, B, P).transpose(0, 2, 1)

    # attention vectors as [HU, 2H]: A[h*U+u, h] = att_w1[h,0,u]; +H for w2
    A12 = np.zeros((HU, 2 * HEADS), dtype=np.float32)
    for h in range(HEADS):
        A12[h * UNITS:(h + 1) * UNITS, h] = att_w1[h, 0]
        A12[h * UNITS:(h + 1) * UNITS, HEADS + h] = att_w2[h, 0]

    npad = -(-n_nodes // XTILE) * XTILE
    xT = np.zeros((in_feat, npad), dtype=np.float32)
    xT[:, :n_nodes] = np.ascontiguousarray(x.T)

    iota = np.broadcast_to(np.arange(P, dtype=np.float16), (P, P)).copy()

    host = {
        "windows": windows,
        "assign": assign,
        "nw": nw, "L": L, "H": H, "B": B, "lo_pos": lo_pos,
        "npad": npad,
        "deg": np.bincount(rcv, minlength=n_nodes),
    }
    Wf = np.ascontiguousarray(W.astype(np.float32))
    WTf = np.ascontiguousarray(Wf.T)
    per_core = []
    for c in range(n_cores):
        sl = np.concatenate(
            [_wrap_idx(snd_lo[i] if i is not None else empty_sl)
             for i in assign[c]], axis=1)
        sh = np.concatenate(
            [_wrap_idx(snd_hi[i] if i is not None else empty_sh)
             for i in assign[c]], axis=1)
        rv = np.concatenate(
            [_wrap_idx(rcv_ix[i] if i is not None else empty_rv)
             for i in assign[c]], axis=1)
        rl = np.concatenate(
            [(rcv_loc_dev[i] if i is not None else empty_rl)
             for i in assign[c]], axis=1)
        per_core.append({
            "xT": xT,
            "W": Wf,
            "WT": WTf,
            "A12": A12,
            "iota": iota,
            "snd_lo": np.ascontiguousarray(sl),
            "snd_hi": np.ascontiguousarray(sh),
            "rcv_ix": np.ascontiguousarray(rv),
            "rcv_loc": np.ascontiguousarray(rl),
        })
    return host, per_core


def _build_bass(n_nodes, npad, nw, in_feat, L, H, B, lo_pos):
    from concourse import bacc, mybir, tile, library_config

    f32 = mybir.dt.float32
    f16 = mybir.dt.float16
    i16 = mybir.dt.int16

    nc = bacc.Bacc("TRN2", target_bir_lowering=False, debug=False,
                   enable_asserts=False, num_devices=1,
                   num_swdge_queues=4)

    xT_d = nc.dram_tensor("xT", [in_feat, npad], f32, kind="ExternalInput").ap()
    W_d = nc.dram_tensor("W", [in_feat, HU], f32, kind="ExternalInput").ap()
    WT_d = nc.dram_tensor("WT", [HU, in_feat], f32, kind="ExternalInput").ap()
    A12_d = nc.dram_tensor("A12", [HU, 2 * HEADS], f32, kind="ExternalInput").ap()
    iota_d = nc.dram_tensor("iota", [P, P], f16, kind="ExternalInput").ap()
    sl_d = nc.dram_tensor("snd_lo", [P, nw * L * 8], i16, kind="ExternalInput").ap()
    sh_d = nc.dram_tensor("snd_hi", [P, nw * H * 8], i16, kind="ExternalInput").ap()
    rv_d = nc.dram_tensor("rcv_ix", [P, nw * B * 8], i16, kind="ExternalInput").ap()
    rl_d = nc.dram_tensor("rcv_loc", [P, nw * B], f16, kind="ExternalInput").ap()

    out_d = nc.dram_tensor("staged", [nw * P, HU], f32, kind="ExternalOutput").ap()

    has_hi = npad > SPLIT
    tlo_d = nc.dram_tensor(
        "t_lo", [min(SPLIT, npad), ROW], f16, kind="Internal").ap()
    thi_d = (nc.dram_tensor("t_hi", [npad - SPLIT, ROW], f16,
                            kind="Internal").ap() if has_hi else tlo_d)

    ntiles = npad // XTILE
    lo_tiles = min(SPLIT, npad) // XTILE
    qctr = [0]

    def next_q():
        q = qctr[0] % 4
        qctr[0] += 1
        return q

    with tile.TileContext(nc) as tc:
        with tc.tile_pool(name="consts", bufs=1) as cpool:
            nc.gpsimd.load_library(library_config.mlp)
            W_sb = cpool.tile([in_feat, HU], f32, tag="w")
            nc.sync.dma_start(out=W_sb[:], in_=W_d[:])
            WT_sb = cpool.tile([HU, in_feat], f32, tag="wt")
            nc.sync.dma_start(out=WT_sb[:], in_=WT_d[:])
            A12_sb = cpool.tile([HU, 2 * HEADS], f32, tag="a12")
            nc.sync.dma_start(out=A12_sb[:], in_=A12_d[:])
            iota_sb = cpool.tile([P, P], f16, tag="iota")
            nc.sync.dma_start(out=iota_sb[:], in_=iota_d[:])
            sl_sb = cpool.tile([P, nw * L * 8], i16, tag="sl")
            nc.sync.dma_start(out=sl_sb[:], in_=sl_d[:])
            sh_sb = cpool.tile([P, nw * H * 8], i16, tag="sh")
            nc.sync.dma_start(out=sh_sb[:], in_=sh_d[:])
            rv_sb = cpool.tile([P, nw * B * 8], i16, tag="rv")
            nc.sync.dma_start(out=rv_sb[:], in_=rv_d[:])
            rl_sb = cpool.tile([P, nw * B], f16, tag="rl")
            nc.sync.dma_start(out=rl_sb[:], in_=rl_d[:])
            wcat_sb = cpool.tile([in_feat, WC], f32, tag="wcat")

            # wcat = [W/HSCALE | W@A12]
            with tc.tile_pool(name="p0psum", bufs=1, space="PSUM") as p0:
                wa_ps = p0.tile([in_feat, 2 * HEADS], f32, tag="wa")
                nc.tensor.matmul(out=wa_ps[:], lhsT=WT_sb[:], rhs=A12_sb[:],
                                 start=True, stop=True)
                nc.vector.tensor_copy(out=wcat_sb[:, HU:], in_=wa_ps[:])
                nc.vector.tensor_scalar(
                    out=wcat_sb[:, :HU], in0=W_sb[:],
                    scalar1=1.0 / HSCALE, scalar2=0.0,
                    op0=mybir.AluOpType.mult, op1=mybir.AluOpType.add)

            # ---- phase A: hs tables ----
            nblk = XTILE // P
            with tc.tile_pool(name="pa_x", bufs=3) as pax, \
                 tc.tile_pool(name="pa_ps", bufs=2, space="PSUM") as paps, \
                 tc.tile_pool(name="pa_hs", bufs=3) as pahs:
              for _rep in range(REPS):
                for t in range(ntiles):
                    xt = pax.tile([in_feat, XTILE], f32, tag="xt")
                    nc.sync.dma_start(
                        out=xt[:], in_=xT_d[:, t * XTILE:(t + 1) * XTILE])
                    ps = paps.tile([P, nblk * WC], f32, tag="ps")
                    for i in range(nblk):
                        nc.tensor.matmul(
                            out=ps[:, i * WC:(i + 1) * WC],
                            lhsT=xt[:, i * P:(i + 1) * P],
                            rhs=wcat_sb[:], start=True, stop=True)
                    hst = pahs.tile([P, nblk * ROW], f16, tag="hst")
                    nc.vector.memset(hst[:], 0.0)
                    for i in range(nblk):
                        nc.vector.tensor_copy(
                            out=hst[:, i * ROW:i * ROW + WC],
                            in_=ps[:, i * WC:(i + 1) * WC])
                    if t < lo_tiles:
                        dst = tlo_d[t * XTILE:(t + 1) * XTILE, :]
                    else:
                        dst = thi_d[(t - lo_tiles) * XTILE:
                                    (t - lo_tiles + 1) * XTILE, :]
                    nc.sync.dma_start(
                        out=dst.rearrange("(i p) c -> p i c", p=P),
                        in_=hst[:].rearrange("p (i c) -> p i c", c=ROW))

            # ---- phase B: windows ----
            with tc.tile_pool(name="pb_g", bufs=3) as pbg, \
                 tc.tile_pool(name="pb_sm", bufs=2) as pbsm, \
                 tc.tile_pool(name="pb_oh", bufs=2) as pboh, \
                 tc.tile_pool(name="pb_ps", bufs=2, space="PSUM") as pbps, \
                 tc.tile_pool(name="pb_out", bufs=2) as pbout:
              for _rep in range(REPS):
                for w in range(nw if ABLATE != "phaseA" else 0):
                    rtab = tlo_d if w < lo_pos else thi_d
                    hs_g = pbg.tile([P, B * ROW], f16, tag="hsg")
                    s2_g = pbg.tile([P, B * ROW], f16, tag="s2g")
                    if ABLATE != "nogather":
                        nc.gpsimd.dma_gather(
                            hs_g[:, :L * ROW].rearrange(
                                "p (b e) -> p b e", e=ROW),
                            tlo_d[:],
                            sl_sb[:, w * L * 8:(w + 1) * L * 8],
                            L * P, L * P, ROW,
                            single_packet=False, queue_num=next_q())
                        nc.gpsimd.dma_gather(
                            hs_g[:, L * ROW:].rearrange(
                                "p (b e) -> p b e", e=ROW),
                            thi_d[:],
                            sh_sb[:, w * H * 8:(w + 1) * H * 8],
                            H * P, H * P, ROW,
                            single_packet=False, queue_num=next_q())
                        nc.gpsimd.dma_gather(
                            s2_g[:].rearrange("p (b e) -> p b e", e=ROW),
                            rtab[:],
                            rv_sb[:, w * B * 8:(w + 1) * B * 8],
                            B * P, B * P, ROW,
                            single_packet=False, queue_num=next_q())
                    if ABLATE == "nocompute":
                        continue

                    hs_g3 = hs_g[:].rearrange("p (b e) -> p b e", e=ROW)
                    s2_g3 = s2_g[:].rearrange("p (b e) -> p b e", e=ROW)
                    logit = pbsm.tile([P, B * HEADS], f16, tag="logit")
                    lg3 = logit[:].rearrange("p (b h) -> p b h", h=HEADS)
                    nc.vector.tensor_add(
                        out=lg3, in0=hs_g3[:, :, HU:HU + HEADS],
                        in1=s2_g3[:, :, HU + HEADS:WC])
                    neg = pbsm.tile([P, B * HEADS], f16, tag="neg")
                    nc.vector.tensor_scalar(
                        out=neg[:], in0=logit[:], scalar1=0.0,
                        scalar2=LEAKY_ALPHA, op0=mybir.AluOpType.min,
                        op1=mybir.AluOpType.mult)
                    lrl = pbsm.tile([P, B * HEADS], f16, tag="lrl")
                    nc.vector.scalar_tensor_tensor(
                        out=lrl[:], in0=logit[:], scalar=0.0, in1=neg[:],
                        op0=mybir.AluOpType.max, op1=mybir.AluOpType.add)
                    lrl2 = pbsm.tile([P, B * HEADS], f16, tag="lrl2")
                    nc.vector.tensor_scalar(
                        out=lrl2[:], in0=lrl[:], scalar1=CLAMP, scalar2=0.0,
                        op0=mybir.AluOpType.min, op1=mybir.AluOpType.add)
                    expo = pbsm.tile([P, B * HEADS], f16, tag="expo")
                    nc.scalar.activation(
                        out=expo[:], in_=lrl2[:],
                        func=mybir.ActivationFunctionType.Exp)

                    rhs = pbg.tile([P, B * (HU + HEADS)], f16, tag="rhs")
                    rhs3 = rhs[:].rearrange("p (b c) -> p b c", c=HU + HEADS)
                    ex3 = expo[:].rearrange("p (b h) -> p b h", h=HEADS)
                    nc.vector.tensor_tensor(
                        out=rhs3[:, :, 0:HU].rearrange(
                            "p b (h u) -> p b h u", u=UNITS),
                        in0=hs_g3[:, :, 0:HU].rearrange(
                            "p b (h u) -> p b h u", u=UNITS),
                        in1=ex3.broadcast_to([P, B, HEADS, UNITS]),
                        op=mybir.AluOpType.mult)
                    nc.vector.tensor_copy(out=rhs3[:, :, HU:HU + HEADS], in_=ex3)

                    onehot = pboh.tile([P, B * P], f16, tag="oh")
                    oh3 = onehot[:].rearrange("p (b c) -> p b c", c=P)
                    nc.vector.tensor_tensor(
                        out=oh3,
                        in0=iota_sb[:].broadcast_to([P, P, B]).rearrange(
                            "p c b -> p b c"),
                        in1=rl_sb[:, w * B:(w + 1) * B].broadcast_to([P, B, P]),
                        op=mybir.AluOpType.is_equal)

                    ps = pbps.tile([P, HU + HEADS], f32, tag="acc")
                    for b in range(B):
                        nc.tensor.matmul(
                            out=ps[:],
                            lhsT=onehot[:, b * P:(b + 1) * P],
                            rhs=rhs[:, b * (HU + HEADS):(b + 1) * (HU + HEADS)],
                            start=(b == 0), stop=(b == B - 1))

                    recip = pbout.tile([P, HEADS], f32, tag="recip")
                    nc.vector.reciprocal(out=recip[:], in_=ps[:, HU:HU + HEADS])
                    recip64 = pbout.tile([P, HEADS], f32, tag="recip64")
                    nc.vector.tensor_scalar(
                        out=recip64[:], in0=recip[:], scalar1=HSCALE, scalar2=0.0,
                        op0=mybir.AluOpType.mult, op1=mybir.AluOpType.add)
                    osb = pbout.tile([P, HU], f32, tag="osb")
                    nc.vector.tensor_tensor(
                        out=osb[:].rearrange("p (h u) -> p h u", u=UNITS),
                        in0=ps[:, 0:HU].rearrange("p (h u) -> p h u", u=UNITS),
                        in1=recip64[:].broadcast_to([P, HEADS, UNITS]),
                        op=mybir.AluOpType.mult)
                    nc.sync.dma_start(
                        out=out_d[w * P:(w + 1) * P, :], in_=osb[:])

    nc.compile()
    return nc


def _run(nc, per_core, n_cores):
    from concourse import bass_utils

    try:
        return bass_utils.run_bass_kernel_spmd(
            nc, per_core, core_ids=list(range(n_cores)))
    except Exception:
        # one retry: a previously wedged device context can fail the first
        # attempt with NRT_EXEC_UNIT_UNRECOVERABLE and recover on re-run
        return bass_utils.run_bass_kernel_spmd(
            nc, per_core, core_ids=list(range(n_cores)))


def _assemble(host, results, n_nodes):
    out = np.zeros((n_nodes, HU), dtype=np.float32)
    for c, row in enumerate(host["assign"]):
        staged = results[c]["staged"]
        for s, i in enumerate(row):
            if i is None:
                continue
            n0, cnt, _, _ = host["windows"][i]
            out[n0:n0 + cnt] = staged[s * P:s * P + cnt]
    out[host["deg"] == 0] = 0.0
    return out


def kernel(x, edge_index, W, att_w1, att_w2, n_cores=8, _return_results=False):
    x = np.asarray(x)
    edge_index = np.asarray(edge_index)
    W = np.asarray(W)
    att_w1 = np.asarray(att_w1)
    att_w2 = np.asarray(att_w2)

    n_nodes, in_feat = x.shape
    host, per_core = _build_host_data(x, edge_index, W, att_w1, att_w2, n_cores)
    nc = _build_bass(n_nodes, host["npad"], host["nw"], in_feat,
                     host["L"], host["H"], host["B"], host["lo_pos"])
    res = _run(nc, per_core, n_cores)
    out = _assemble(host, res.results, n_nodes)
    if _return_results:
        return out, res
    return out
